# revision 5
# baseline (speedup 1.0000x reference)
"""Trainium2 Bass kernel for a basis-customized linear layer.

Reference computation (B=1024, IN=OUT=512, EMB=64, KQ=64, NB=3, VOCAB=100):
    embs = concat(emb_author[idx_author], emb_citation[idx_citation])  # [B, 128]
    h    = tanh(embs @ W1.T + b1)                                      # [B, 64]
    coef = softmax(h @ W2.T)                                           # [B, 3]
    w    = (coef @ W3.T + b3).reshape(B, IN, OUT)
    out  = einsum('bi,bio->bo', x, w)                                  # [B, 512]

Key rewrite: w[b] = sum_k coef[b,k]*W3k + b3r, so
    out = sum_k coef[:,k] * (x @ W3k) + x @ b3r
i.e. 4 shared [512,512] matmuls + a per-sample 4-way weighted combine.

Sharding over 8 cores: batch 2-way x out-column 4-way. Each core gets
x.T[:, batch_half] (1MB) and the 4 weight column-blocks for its 128 output
columns (1MB), computes coef for its 512 rows on-device (embedding gather via
one-hot matmul, fused with W1), and writes a [512, 128] output block.
"""

import numpy as np
import ml_dtypes

import concourse.bass as bass
import concourse.tile as tile
from concourse import bacc, mybir
from concourse.bass_utils import run_bass_kernel_spmd

# Problem dims (hardcoded per contract)
B, IN, OUT = 1024, 512, 512
EMB, KQ, NB, VOCAB = 64, 64, 3, 100
P_B, Q_O = 2, 4            # batch shards x out-col shards = 8 cores
BS = B // P_B              # 512 batch rows per core
OS = OUT // Q_O            # 128 out cols per core
NBLK = NB + 1              # 3 basis blocks + 1 bias block
KT = IN // 128             # 4 contraction tiles
MT = BS // 128             # 4 batch tiles per core

F32 = mybir.dt.float32
F32R = mybir.dt.float32r
BF16 = mybir.dt.bfloat16

# Dtype knobs (precision vs PE speed)
Y_USE_F32R = True          # big x@W matmuls in f32r (1 cyc/row vs 4)
GATHER_USE_F32R = False    # one-hot gather matmuls
Y_DT = F32R if Y_USE_F32R else F32
G_DT = F32R if GATHER_USE_F32R else F32

LAST_RESULT = None         # BassKernelResults of the most recent run (for test.py)

_NC_CACHE = None


def _build_nc():
    nc = bacc.Bacc("TRN2", target_bir_lowering=False, debug=False,
                   num_devices=P_B * Q_O)

    xt = nc.dram_tensor("xt", [IN, BS], Y_DT, kind="ExternalInput")
    wc = nc.dram_tensor("wc", [IN, NBLK * OS], Y_DT, kind="ExternalInput")
    emba_t = nc.dram_tensor("emba_t", [EMB, VOCAB], F32, kind="ExternalInput")
    embc_t = nc.dram_tensor("embc_t", [EMB, VOCAB], F32, kind="ExternalInput")
    w1a_t = nc.dram_tensor("w1a_t", [EMB, KQ], F32, kind="ExternalInput")
    w1c_t = nc.dram_tensor("w1c_t", [EMB, KQ], F32, kind="ExternalInput")
    w2t = nc.dram_tensor("w2t", [KQ, NB], F32, kind="ExternalInput")
    b1v = nc.dram_tensor("b1v", [KQ, 1], F32, kind="ExternalInput")
    bw = nc.dram_tensor("bw", [2, VOCAB], BF16, kind="ExternalInput")
    idxa = nc.dram_tensor("idxa", [2, BS], BF16, kind="ExternalInput")
    idxc = nc.dram_tensor("idxc", [2, BS], BF16, kind="ExternalInput")
    out = nc.dram_tensor("out", [BS, OS], F32, kind="ExternalOutput")

    with tile.TileContext(nc) as tc:
        with (
            tc.tile_pool(name="consts", bufs=1) as consts,
            tc.tile_pool(name="xw", bufs=1) as xw,
            tc.tile_pool(name="work", bufs=3) as work,
            tc.tile_pool(name="ps_idx", bufs=2, space="PSUM") as ps_idx,
            tc.tile_pool(name="ps_pre", bufs=2, space="PSUM") as ps_pre,
            tc.tile_pool(name="ps_y", bufs=4, space="PSUM") as ps_y,
        ):
            # ---- small input loads ----
            idxa_sb = consts.tile([2, BS], BF16)
            nc.sync.dma_start(out=idxa_sb, in_=idxa[:, :])
            idxc_sb = consts.tile([2, BS], BF16)
            nc.sync.dma_start(out=idxc_sb, in_=idxc[:, :])
            bw_sb = consts.tile([2, VOCAB], BF16)
            nc.sync.dma_start(out=bw_sb, in_=bw[:, :])
            emba_sb = consts.tile([EMB, VOCAB], F32)
            nc.sync.dma_start(out=emba_sb, in_=emba_t[:, :])
            embc_sb = consts.tile([EMB, VOCAB], F32)
            nc.sync.dma_start(out=embc_sb, in_=embc_t[:, :])
            w1a_sb = consts.tile([EMB, KQ], F32)
            nc.sync.dma_start(out=w1a_sb, in_=w1a_t[:, :])
            w1c_sb = consts.tile([EMB, KQ], F32)
            nc.sync.dma_start(out=w1c_sb, in_=w1c_t[:, :])
            w2t_sb = consts.tile([KQ, NB], F32)
            nc.sync.dma_start(out=w2t_sb, in_=w2t[:, :])
            b1_ld = consts.tile([KQ, 1], F32)
            nc.sync.dma_start(out=b1_ld, in_=b1v[:, :])
            # route b1 through ACT so Tanh's bias dep is same-engine
            b1_sb = consts.tile([KQ, 1], F32)
            nc.scalar.copy(out=b1_sb, in_=b1_ld)

            # ---- per-vocab fused gather tables: G = emb @ W1half.T  [VOCAB, KQ]
            gat_ps = ps_pre.tile([VOCAB, KQ], F32, tag="pre")
            nc.tensor.matmul(gat_ps, lhsT=emba_sb, rhs=w1a_sb, start=True, stop=True)
            gat_sb = consts.tile([VOCAB, KQ], G_DT)
            nc.vector.tensor_copy(out=gat_sb, in_=gat_ps)
            gct_ps = ps_pre.tile([VOCAB, KQ], F32, tag="pre")
            nc.tensor.matmul(gct_ps, lhsT=embc_sb, rhs=w1c_sb, start=True, stop=True)
            gct_sb = consts.tile([VOCAB, KQ], G_DT)
            nc.vector.tensor_copy(out=gct_sb, in_=gct_ps)

            # ---- stage A: coef for all BS rows ----
            # broadcast idx across VOCAB partitions via K=1 matmul of ones
            bca_ps = ps_idx.tile([VOCAB, BS], F32, tag="idx")
            nc.tensor.matmul(bca_ps, lhsT=bw_sb, rhs=idxa_sb, start=True, stop=True)
            oha_sb = consts.tile([VOCAB, BS], G_DT)
            nc.vector.tensor_scalar(
                out=oha_sb, in0=bca_ps, scalar1=0.0, scalar2=None,
                op0=mybir.AluOpType.is_equal,
            )
            bcc_ps = ps_idx.tile([VOCAB, BS], F32, tag="idx")
            nc.tensor.matmul(bcc_ps, lhsT=bw_sb, rhs=idxc_sb, start=True, stop=True)
            ohc_sb = consts.tile([VOCAB, BS], G_DT)
            nc.vector.tensor_scalar(
                out=ohc_sb, in0=bcc_ps, scalar1=0.0, scalar2=None,
                op0=mybir.AluOpType.is_equal,
            )

            # fused gather + W1: preact.T [KQ, BS]
            pre_ps = ps_pre.tile([KQ, BS], F32, tag="pre")
            nc.tensor.matmul(
                pre_ps, lhsT=gat_sb, rhs=oha_sb, start=True, stop=False,
            )
            nc.tensor.matmul(
                pre_ps, lhsT=gct_sb, rhs=ohc_sb, start=False, stop=True,
            )
            ht_sb = consts.tile([KQ, BS], F32)
            nc.scalar.activation(
                out=ht_sb, in_=pre_ps, func=mybir.ActivationFunctionType.Tanh,
                bias=b1_sb, scale=1.0,
            )

            # softmax over NB bases, per 128-row tile
            coef_sb = []
            for m in range(MT):
                lg_ps = ps_pre.tile([128, NB], F32, tag="pre")
                nc.tensor.matmul(
                    lg_ps, lhsT=ht_sb[:, m * 128:(m + 1) * 128], rhs=w2t_sb,
                    start=True, stop=True,
                )
                e_sb = work.tile([128, NB], F32, tag="e")
                nc.scalar.activation(
                    out=e_sb, in_=lg_ps, func=mybir.ActivationFunctionType.Exp,
                )
                s_sb = work.tile([128, 1], F32, tag="s")
                nc.vector.reduce_sum(out=s_sb, in_=e_sb, axis=mybir.AxisListType.X)
                r_sb = work.tile([128, 1], F32, tag="r")
                nc.vector.reciprocal(out=r_sb, in_=s_sb)
                cf = consts.tile([128, NB], F32, name=f"coef{m}", tag=f"coef{m}")
                nc.vector.tensor_scalar(
                    out=cf, in0=e_sb, scalar1=r_sb, scalar2=None,
                    op0=mybir.AluOpType.mult,
                )
                coef_sb.append(cf)

            # ---- big loads: x.T and weight-column blocks, per k-tile ----
            xk_sb, wk_sb = [], []
            for k in range(KT):
                w_t = xw.tile([128, NBLK * OS], Y_DT, tag=f"wk{k}")
                nc.sync.dma_start(out=w_t, in_=wc[k * 128:(k + 1) * 128, :])
                x_t = xw.tile([128, BS], Y_DT, tag=f"xk{k}")
                nc.sync.dma_start(out=x_t, in_=xt[k * 128:(k + 1) * 128, :])
                wk_sb.append(w_t)
                xk_sb.append(x_t)

            # ---- stage B: Y[m] = x_tile @ Wc, accumulated over k ----
            y_ps = [
                ps_y.tile([128, NBLK * OS], F32, tag="y", name=f"y{m}")
                for m in range(MT)
            ]
            for k in range(KT):
                for m in range(MT):
                    nc.tensor.matmul(
                        y_ps[m],
                        lhsT=xk_sb[k][:, m * 128:(m + 1) * 128],
                        rhs=wk_sb[k],
                        start=(k == 0), stop=(k == KT - 1),
                    )

            # ---- combine: out = sum_j coef_j * Y_j + Y_bias, then store ----
            for m in range(MT):
                acc = work.tile([128, OS], F32, tag="acc")
                nc.vector.tensor_scalar(
                    out=acc, in0=y_ps[m][:, 0:OS], scalar1=coef_sb[m][:, 0:1],
                    scalar2=None, op0=mybir.AluOpType.mult,
                )
                for j in (1, 2):
                    tmp = work.tile([128, OS], F32, tag="tmp")
                    nc.vector.tensor_scalar(
                        out=tmp, in0=y_ps[m][:, j * OS:(j + 1) * OS],
                        scalar1=coef_sb[m][:, j:j + 1], scalar2=None,
                        op0=mybir.AluOpType.mult,
                    )
                    nc.vector.tensor_tensor(
                        out=acc, in0=acc, in1=tmp, op=mybir.AluOpType.add,
                    )
                nc.vector.tensor_tensor(
                    out=acc, in0=acc, in1=y_ps[m][:, NB * OS:NBLK * OS],
                    op=mybir.AluOpType.add,
                )
                nc.sync.dma_start(out=out[m * 128:(m + 1) * 128, :], in_=acc)

    nc.compile()
    return nc


def _get_nc():
    global _NC_CACHE
    if _NC_CACHE is None:
        _NC_CACHE = _build_nc()
    return _NC_CACHE


def _make_in_maps(x, idx_author, idx_citation, emb_author, emb_citation,
                  W1, b1, W2, W3, b3):
    f = np.float32
    xt_full = np.ascontiguousarray(np.asarray(x, dtype=f).T)          # [IN, B]
    W3r = np.asarray(W3, dtype=f).reshape(IN, OUT, NB)
    b3r = np.asarray(b3, dtype=f).reshape(IN, OUT)
    W1 = np.asarray(W1, dtype=f)
    shared = {
        "emba_t": np.ascontiguousarray(np.asarray(emb_author, dtype=f).T),
        "embc_t": np.ascontiguousarray(np.asarray(emb_citation, dtype=f).T),
        "w1a_t": np.ascontiguousarray(W1[:, :EMB].T),
        "w1c_t": np.ascontiguousarray(W1[:, EMB:].T),
        "w2t": np.ascontiguousarray(np.asarray(W2, dtype=f).T),
        "b1v": np.asarray(b1, dtype=f).reshape(KQ, 1),
        "bw": np.stack([np.ones(VOCAB, f), -np.arange(VOCAB, dtype=f)]
                       ).astype(ml_dtypes.bfloat16),
    }
    ia = np.asarray(idx_author).astype(ml_dtypes.bfloat16)
    ic = np.asarray(idx_citation).astype(ml_dtypes.bfloat16)
    ones_bs = np.ones((BS,), ml_dtypes.bfloat16)

    wc_blocks = []
    for oj in range(Q_O):
        cols = slice(oj * OS, (oj + 1) * OS)
        blk = np.moveaxis(W3r[:, cols, :], 2, 1).reshape(IN, NB * OS)  # [IN, 3*OS]
        wc_blocks.append(
            np.ascontiguousarray(np.concatenate([blk, b3r[:, cols]], axis=1))
        )

    in_maps = []
    for c in range(P_B * Q_O):
        bi, oj = c // Q_O, c % Q_O
        rows = slice(bi * BS, (bi + 1) * BS)
        in_maps.append({
            "xt": np.ascontiguousarray(xt_full[:, rows]),
            "wc": wc_blocks[oj],
            "idxa": np.stack([ia[rows], ones_bs]),
            "idxc": np.stack([ic[rows], ones_bs]),
            **shared,
        })
    return in_maps


def kernel(x, idx_author, idx_citation, emb_author, emb_citation,
           W1, b1, W2, W3, b3):
    global LAST_RESULT
    nc = _get_nc()
    in_maps = _make_in_maps(x, idx_author, idx_citation, emb_author,
                            emb_citation, W1, b1, W2, W3, b3)
    res = run_bass_kernel_spmd(nc, in_maps, core_ids=list(range(P_B * Q_O)))
    LAST_RESULT = res
    out = np.empty((B, OUT), dtype=np.float32)
    for c in range(P_B * Q_O):
        bi, oj = c // Q_O, c % Q_O
        out[bi * BS:(bi + 1) * BS, oj * OS:(oj + 1) * OS] = res.results[c]["out"]
    return out


# revision 6
# speedup vs baseline: 1.0988x; 1.0988x over previous
"""Trainium2 Bass kernel for a basis-customized linear layer.

Reference computation (B=1024, IN=OUT=512, EMB=64, KQ=64, NB=3, VOCAB=100):
    embs = concat(emb_author[idx_author], emb_citation[idx_citation])  # [B, 128]
    h    = tanh(embs @ W1.T + b1)                                      # [B, 64]
    coef = softmax(h @ W2.T)                                           # [B, 3]
    w    = (coef @ W3.T + b3).reshape(B, IN, OUT)
    out  = einsum('bi,bio->bo', x, w)                                  # [B, 512]

Key rewrites:
  (1) w[b] = sum_j coef[b,j]*W3j + b3r, so
      out = sum_j coef[:,j] * (x @ W3j) + x @ b3r
      -- 3 shared [512,512] matmuls + a per-sample weighted combine, instead
      of materializing the 1GB per-sample weight.
  (2) since softmax coefs sum to 1, the bias folds into every basis block:
      out = sum_j coef[:,j] * (x @ (W3j + b3r))
  (3) the embedding gather is a one-hot matmul (built from an iota-compare),
      fused with W1 via the precomputed per-vocab table G = emb @ W1half.T.

Sharding over 8 cores: batch 2-way x out-column 4-way. Each core gets
x.T[:, batch_half] (1MB) and its 3 basis column-blocks (0.75MB), computes
coef for its 512 rows on-device, and writes a [512, 128] output block.
All inputs are packed host-side into partition-major [128, *] layouts so each
is a single descriptor-per-partition DMA.
"""

import numpy as np
import ml_dtypes

import concourse.bass as bass
import concourse.tile as tile
from concourse import bacc, mybir
from concourse.bass_utils import run_bass_kernel_spmd

# Problem dims (hardcoded per contract)
B, IN, OUT = 1024, 512, 512
EMB, KQ, NB, VOCAB = 64, 64, 3, 100
P_B, Q_O = 2, 4            # batch shards x out-col shards = 8 cores
BS = B // P_B              # 512 batch rows per core
OS = OUT // Q_O            # 128 out cols per core
KT = IN // 128             # 4 contraction tiles
MT = BS // 128             # 4 batch tiles per core

F32 = mybir.dt.float32
F32R = mybir.dt.float32r
BF16 = mybir.dt.bfloat16

N_WARMUP_MM = 8            # dummy matmuls to warm the PE HAM clock gate
SB16 = 2 * BS + VOCAB      # packed bf16 input columns
SF32 = 2 * VOCAB + 2 * KQ + NB + 1  # packed f32 small-input columns

LAST_RESULT = None         # BassKernelResults of the most recent run (for test.py)

_NC_CACHE = None


def _build_nc():
    nc = bacc.Bacc("TRN2", target_bir_lowering=False, debug=False,
                   num_devices=P_B * Q_O)

    xt = nc.dram_tensor("xt", [128, KT * BS], F32R, kind="ExternalInput")
    wc = nc.dram_tensor("wc", [128, KT * NB * OS], F32R, kind="ExternalInput")
    sf = nc.dram_tensor("sf", [EMB, SF32], F32, kind="ExternalInput")
    sb = nc.dram_tensor("sb", [2, SB16], BF16, kind="ExternalInput")
    out = nc.dram_tensor("out", [128, MT * OS], F32, kind="ExternalOutput")

    with tile.TileContext(nc) as tc:
        with (
            tc.tile_pool(name="consts", bufs=1) as consts,
            tc.tile_pool(name="work", bufs=3) as work,
            tc.tile_pool(name="ps_idx", bufs=2, space="PSUM") as ps_idx,
            tc.tile_pool(name="ps_pre", bufs=2, space="PSUM") as ps_pre,
            tc.tile_pool(name="ps_y", bufs=4, space="PSUM") as ps_y,
        ):
            # ---- PE warmup: keep TensorE busy so the HAM clock ungates early
            dum_sb = consts.tile([2, BS], BF16)
            nc.vector.memset(dum_sb, 1.0)
            for w in range(N_WARMUP_MM):
                wu_ps = ps_idx.tile([VOCAB, BS], F32, tag="idx", name=f"wu{w}")
                nc.tensor.matmul(wu_ps, lhsT=dum_sb[:, :VOCAB], rhs=dum_sb,
                                 start=True, stop=True)

            # ---- packed input loads (one descriptor-per-partition each) ----
            sb_sb = consts.tile([2, SB16], BF16)
            nc.sync.dma_start(out=sb_sb, in_=sb[:, :])
            sf_sb = consts.tile([EMB, SF32], F32)
            nc.scalar.dma_start(out=sf_sb, in_=sf[:, :])
            xall = consts.tile([128, KT, BS], F32R)
            nc.sync.dma_start(out=xall,
                              in_=xt[:, :].rearrange("p (k n) -> p k n", k=KT))
            wall = consts.tile([128, KT, NB * OS], F32R)
            nc.scalar.dma_start(out=wall,
                                in_=wc[:, :].rearrange("p (k n) -> p k n", k=KT))

            idxa_sb = sb_sb[:, 0:BS]
            idxc_sb = sb_sb[:, BS:2 * BS]
            bw_sb = sb_sb[:, 2 * BS:2 * BS + VOCAB]
            emba_sb = sf_sb[:, 0:VOCAB]
            embc_sb = sf_sb[:, VOCAB:2 * VOCAB]
            w1a_sb = sf_sb[:, 2 * VOCAB:2 * VOCAB + KQ]
            w1c_sb = sf_sb[:, 2 * VOCAB + KQ:2 * VOCAB + 2 * KQ]
            w2t_sb = sf_sb[:, 2 * VOCAB + 2 * KQ:2 * VOCAB + 2 * KQ + NB]

            # b1 routed through ACT so Tanh's bias dep is same-engine
            b1_sb = consts.tile([KQ, 1], F32)
            nc.scalar.copy(out=b1_sb, in_=sf_sb[:, SF32 - 1:SF32])

            # ---- per-vocab fused gather tables: G = emb @ W1half.T [VOCAB, KQ]
            gat_ps = ps_pre.tile([VOCAB, KQ], F32, tag="pre")
            nc.tensor.matmul(gat_ps, lhsT=emba_sb, rhs=w1a_sb, start=True, stop=True)
            gat_sb = consts.tile([VOCAB, KQ], F32R)
            nc.vector.tensor_copy(out=gat_sb, in_=gat_ps)
            gct_ps = ps_pre.tile([VOCAB, KQ], F32, tag="pre")
            nc.tensor.matmul(gct_ps, lhsT=embc_sb, rhs=w1c_sb, start=True, stop=True)
            gct_sb = consts.tile([VOCAB, KQ], F32R)
            nc.vector.tensor_copy(out=gct_sb, in_=gct_ps)

            # ---- stage A: coef for all BS rows ----
            # one-hot via K=2 matmul: psum[v,b] = idx[b]*1 + 1*(-v), then ==0
            bca_ps = ps_idx.tile([VOCAB, BS], F32, tag="idx")
            nc.tensor.matmul(bca_ps, lhsT=bw_sb, rhs=idxa_sb, start=True, stop=True)
            oha_sb = consts.tile([VOCAB, BS], F32R)
            nc.vector.tensor_scalar(
                out=oha_sb, in0=bca_ps, scalar1=0.0, scalar2=None,
                op0=mybir.AluOpType.is_equal,
            )
            bcc_ps = ps_idx.tile([VOCAB, BS], F32, tag="idx")
            nc.tensor.matmul(bcc_ps, lhsT=bw_sb, rhs=idxc_sb, start=True, stop=True)
            ohc_sb = consts.tile([VOCAB, BS], F32R)
            nc.vector.tensor_scalar(
                out=ohc_sb, in0=bcc_ps, scalar1=0.0, scalar2=None,
                op0=mybir.AluOpType.is_equal,
            )

            # fused gather + W1: preact.T [KQ, BS]
            pre_ps = ps_pre.tile([KQ, BS], F32, tag="pre")
            nc.tensor.matmul(pre_ps, lhsT=gat_sb, rhs=oha_sb, start=True, stop=False)
            nc.tensor.matmul(pre_ps, lhsT=gct_sb, rhs=ohc_sb, start=False, stop=True)
            ht_sb = consts.tile([KQ, BS], F32)
            nc.scalar.activation(
                out=ht_sb, in_=pre_ps, func=mybir.ActivationFunctionType.Tanh,
                bias=b1_sb, scale=1.0,
            )

            # softmax over NB bases, per 128-row tile
            coef_sb = []
            for m in range(MT):
                lg_ps = ps_pre.tile([128, NB], F32, tag="pre")
                nc.tensor.matmul(
                    lg_ps, lhsT=ht_sb[:, m * 128:(m + 1) * 128], rhs=w2t_sb,
                    start=True, stop=True,
                )
                e_sb = work.tile([128, NB], F32, tag="e")
                nc.scalar.activation(
                    out=e_sb, in_=lg_ps, func=mybir.ActivationFunctionType.Exp,
                )
                s_sb = work.tile([128, 1], F32, tag="s")
                nc.vector.reduce_sum(out=s_sb, in_=e_sb, axis=mybir.AxisListType.X)
                r_sb = work.tile([128, 1], F32, tag="r")
                nc.vector.reciprocal(out=r_sb, in_=s_sb)
                cf = consts.tile([128, NB], F32, name=f"coef{m}", tag=f"coef{m}")
                nc.vector.tensor_scalar(
                    out=cf, in0=e_sb, scalar1=r_sb, scalar2=None,
                    op0=mybir.AluOpType.mult,
                )
                coef_sb.append(cf)

            # ---- stage B: Y[m][b, o, j] = x_tile @ (W3j + b3r), k-accumulated
            y_ps = [
                ps_y.tile([128, OS, NB], F32, tag="y", name=f"y{m}")
                for m in range(MT)
            ]
            for k in range(KT):
                for m in range(MT):
                    nc.tensor.matmul(
                        y_ps[m],
                        lhsT=xall[:, k, m * 128:(m + 1) * 128],
                        rhs=wall[:, k, :].rearrange("p (o j) -> p o j", j=NB),
                        start=(k == 0), stop=(k == KT - 1),
                    )

            # ---- combine: out[b,o] = sum_j coef[b,j] * Y[b,o,j] ----
            out_sb = consts.tile([128, MT, OS], F32)
            for m in range(MT):
                cb = bass.AP(
                    tensor=coef_sb[m].tensor, offset=coef_sb[m].offset,
                    ap=[list(coef_sb[m].ap[0]), [0, OS], list(coef_sb[m].ap[1])],
                )
                tmp = work.tile([128, OS, NB], F32, tag="tmp")
                nc.vector.tensor_tensor(
                    out=tmp, in0=y_ps[m], in1=cb, op=mybir.AluOpType.mult,
                )
                nc.vector.reduce_sum(
                    out=out_sb[:, m, :], in_=tmp, axis=mybir.AxisListType.X,
                )
            nc.sync.dma_start(out=out[:, :].rearrange("p (m n) -> p m n", m=MT),
                              in_=out_sb)

    nc.compile()
    return nc


def _get_nc():
    global _NC_CACHE
    if _NC_CACHE is None:
        _NC_CACHE = _build_nc()
    return _NC_CACHE


def _make_in_maps(x, idx_author, idx_citation, emb_author, emb_citation,
                  W1, b1, W2, W3, b3):
    f = np.float32
    x = np.asarray(x, dtype=f)
    W3r = np.asarray(W3, dtype=f).reshape(IN, OUT, NB)
    b3r = np.asarray(b3, dtype=f).reshape(IN, OUT)
    W1 = np.asarray(W1, dtype=f)

    # packed f32 smalls [64, 332]: embA.T | embC.T | W1a.T | W1c.T | W2.T | b1
    sf = np.concatenate([
        np.asarray(emb_author, dtype=f).T,
        np.asarray(emb_citation, dtype=f).T,
        W1[:, :EMB].T,
        W1[:, EMB:].T,
        np.asarray(W2, dtype=f).T,
        np.asarray(b1, dtype=f).reshape(KQ, 1),
    ], axis=1)
    sf = np.ascontiguousarray(sf)

    ia = np.asarray(idx_author).astype(ml_dtypes.bfloat16)
    ic = np.asarray(idx_citation).astype(ml_dtypes.bfloat16)
    ones_bs = np.ones((BS,), ml_dtypes.bfloat16)
    bw = np.stack([np.ones(VOCAB, f), -np.arange(VOCAB, dtype=f)]
                  ).astype(ml_dtypes.bfloat16)

    # per out-shard weight blocks, bias folded in, j innermost, k packed
    wc_blocks = []
    for oj in range(Q_O):
        cols = slice(oj * OS, (oj + 1) * OS)
        blk = W3r[:, cols, :] + b3r[:, cols, None]       # [IN, OS, NB]
        blk = blk.reshape(KT, 128, OS * NB).transpose(1, 0, 2)
        wc_blocks.append(np.ascontiguousarray(blk.reshape(128, KT * OS * NB)))

    # x.T per batch shard, k packed: [128, KT*BS]
    xt_shards = []
    for bi in range(P_B):
        xs = x[bi * BS:(bi + 1) * BS, :].T               # [IN, BS]
        xs = xs.reshape(KT, 128, BS).transpose(1, 0, 2)
        xt_shards.append(np.ascontiguousarray(xs.reshape(128, KT * BS)))

    in_maps = []
    for c in range(P_B * Q_O):
        bi, oj = c // Q_O, c % Q_O
        rows = slice(bi * BS, (bi + 1) * BS)
        sb16 = np.concatenate([
            np.stack([ia[rows], ones_bs]),
            np.stack([ic[rows], ones_bs]),
            bw,
        ], axis=1)
        in_maps.append({
            "xt": xt_shards[bi],
            "wc": wc_blocks[oj],
            "sf": sf,
            "sb": np.ascontiguousarray(sb16),
        })
    return in_maps


def kernel(x, idx_author, idx_citation, emb_author, emb_citation,
           W1, b1, W2, W3, b3):
    global LAST_RESULT
    nc = _get_nc()
    in_maps = _make_in_maps(x, idx_author, idx_citation, emb_author,
                            emb_citation, W1, b1, W2, W3, b3)
    res = run_bass_kernel_spmd(nc, in_maps, core_ids=list(range(P_B * Q_O)))
    LAST_RESULT = res
    out = np.empty((B, OUT), dtype=np.float32)
    for c in range(P_B * Q_O):
        bi, oj = c // Q_O, c % Q_O
        blk = res.results[c]["out"].reshape(128, MT, OS).transpose(1, 0, 2)
        out[bi * BS:(bi + 1) * BS, oj * OS:(oj + 1) * OS] = \
            blk.reshape(BS, OS)
    return out


# revision 9
# speedup vs baseline: 1.1925x; 1.0852x over previous
"""Trainium2 Bass kernel for a basis-customized linear layer.

Reference computation (B=1024, IN=OUT=512, EMB=64, KQ=64, NB=3, VOCAB=100):
    embs = concat(emb_author[idx_author], emb_citation[idx_citation])  # [B, 128]
    h    = tanh(embs @ W1.T + b1)                                      # [B, 64]
    coef = softmax(h @ W2.T)                                           # [B, 3]
    w    = (coef @ W3.T + b3).reshape(B, IN, OUT)
    out  = einsum('bi,bio->bo', x, w)                                  # [B, 512]

Key rewrites:
  (1) w[b] = sum_j coef[b,j]*W3j + b3r, so
      out = sum_j coef[:,j] * (x @ W3j) + x @ b3r
      -- 3 shared [512,512] matmuls + a per-sample weighted combine, instead
      of materializing the 1GB per-sample weight.
  (2) since softmax coefs sum to 1, the bias folds into every basis block:
      out = sum_j coef[:,j] * (x @ (W3j + b3r))
  (3) the embedding gather is a one-hot matmul (built from an iota-compare),
      fused with W1 via the precomputed per-vocab table G = emb @ W1half.T.

Sharding over 8 cores: batch 2-way x out-column 4-way. Each core gets
x.T[:, batch_half] (1MB) and its 3 basis column-blocks (0.75MB), computes
coef for its 512 rows on-device, and writes a [512, 128] output block.
All inputs are packed host-side into partition-major [128, *] layouts so each
is a single descriptor-per-partition DMA.
"""

import numpy as np
import ml_dtypes

import concourse.bass as bass
import concourse.tile as tile
from concourse import bacc, mybir
from concourse.bass_utils import run_bass_kernel_spmd

# Problem dims (hardcoded per contract)
B, IN, OUT = 1024, 512, 512
EMB, KQ, NB, VOCAB = 64, 64, 3, 100
P_B, Q_O = 2, 4            # batch shards x out-col shards = 8 cores
BS = B // P_B              # 512 batch rows per core
OS = OUT // Q_O            # 128 out cols per core
KT = IN // 128             # 4 contraction tiles
MT = BS // 128             # 4 batch tiles per core

F32 = mybir.dt.float32
F32R = mybir.dt.float32r
BF16 = mybir.dt.bfloat16

SB16 = 2 * BS + VOCAB      # packed bf16 input columns
SF32 = 2 * VOCAB + 2 * KQ + NB + 1  # packed f32 small-input columns

LAST_RESULT = None         # BassKernelResults of the most recent run (for test.py)

_NC_CACHE = None


def _build_nc():
    nc = bacc.Bacc("TRN2", target_bir_lowering=False, debug=False,
                   num_devices=P_B * Q_O)

    xt = nc.dram_tensor("xt", [128, KT * BS], F32R, kind="ExternalInput")
    wc = nc.dram_tensor("wc", [128, KT * NB * OS], F32R, kind="ExternalInput")
    sf = nc.dram_tensor("sf", [EMB, SF32], F32, kind="ExternalInput")
    sb = nc.dram_tensor("sb", [2, SB16], BF16, kind="ExternalInput")
    out = nc.dram_tensor("out", [128, MT * OS], F32, kind="ExternalOutput")

    with tile.TileContext(nc) as tc:
        with (
            tc.tile_pool(name="consts", bufs=1) as consts,
            tc.tile_pool(name="work", bufs=3) as work,
            tc.tile_pool(name="ps_idx", bufs=2, space="PSUM") as ps_idx,
            tc.tile_pool(name="ps_pre", bufs=2, space="PSUM") as ps_pre,
            tc.tile_pool(name="ps_y", bufs=4, space="PSUM") as ps_y,
        ):
            # ---- packed input loads (one descriptor-per-partition each) ----
            sb_sb = consts.tile([2, SB16], BF16)
            nc.sync.dma_start(out=sb_sb, in_=sb[:, :])
            sf_sb = consts.tile([EMB, SF32], F32)
            nc.scalar.dma_start(out=sf_sb, in_=sf[:, :])
            xall = consts.tile([128, KT, BS], F32R)
            wall = consts.tile([128, KT, NB * OS], F32R)
            xt_k = xt[:, :].rearrange("p (k n) -> p k n", k=KT)
            wc_k = wc[:, :].rearrange("p (k n) -> p k n", k=KT)
            H = KT // 2
            for half in range(2):
                ks = slice(half * H, (half + 1) * H)
                nc.scalar.dma_start(out=wall[:, ks, :], in_=wc_k[:, ks, :])
                nc.sync.dma_start(out=xall[:, ks, :], in_=xt_k[:, ks, :])

            idxa_sb = sb_sb[:, 0:BS]
            idxc_sb = sb_sb[:, BS:2 * BS]
            bw_sb = sb_sb[:, 2 * BS:2 * BS + VOCAB]
            emba_sb = sf_sb[:, 0:VOCAB]
            embc_sb = sf_sb[:, VOCAB:2 * VOCAB]
            w1a_sb = sf_sb[:, 2 * VOCAB:2 * VOCAB + KQ]
            w1c_sb = sf_sb[:, 2 * VOCAB + KQ:2 * VOCAB + 2 * KQ]
            w2t_sb = sf_sb[:, 2 * VOCAB + 2 * KQ:2 * VOCAB + 2 * KQ + NB]

            # b1 routed through ACT so Tanh's bias dep is same-engine
            b1_sb = consts.tile([KQ, 1], F32)
            nc.scalar.copy(out=b1_sb, in_=sf_sb[:, SF32 - 1:SF32])
            w2r_sb = consts.tile([KQ, NB + 1], F32R)
            nc.vector.memset(w2r_sb.bitcast(mybir.dt.uint32), 0)
            nc.vector.tensor_copy(out=w2r_sb[:, 0:NB], in_=w2t_sb)

            # ---- per-vocab fused gather tables: G = emb @ W1half.T [VOCAB, KQ]
            gat_ps = ps_pre.tile([VOCAB, KQ], F32, tag="pre")
            nc.tensor.matmul(gat_ps, lhsT=emba_sb, rhs=w1a_sb, start=True, stop=True)
            gat_sb = consts.tile([VOCAB, KQ], F32R)
            nc.vector.tensor_copy(out=gat_sb, in_=gat_ps)
            gct_ps = ps_pre.tile([VOCAB, KQ], F32, tag="pre")
            nc.tensor.matmul(gct_ps, lhsT=embc_sb, rhs=w1c_sb, start=True, stop=True)
            gct_sb = consts.tile([VOCAB, KQ], F32R)
            nc.vector.tensor_copy(out=gct_sb, in_=gct_ps)

            # ---- stage A: coef for all BS rows ----
            # one-hot via K=2 matmul: psum[v,b] = idx[b]*1 + 1*(-v), then ==0
            bca_ps = ps_idx.tile([VOCAB, BS], F32, tag="idx")
            nc.tensor.matmul(bca_ps, lhsT=bw_sb, rhs=idxa_sb, start=True, stop=True)
            oha_sb = consts.tile([VOCAB, BS], F32R)
            nc.vector.tensor_scalar(
                out=oha_sb, in0=bca_ps, scalar1=0.0, scalar2=None,
                op0=mybir.AluOpType.is_equal,
            )
            bcc_ps = ps_idx.tile([VOCAB, BS], F32, tag="idx")
            nc.tensor.matmul(bcc_ps, lhsT=bw_sb, rhs=idxc_sb, start=True, stop=True)
            ohc_sb = consts.tile([VOCAB, BS], F32R)
            nc.vector.tensor_scalar(
                out=ohc_sb, in0=bcc_ps, scalar1=0.0, scalar2=None,
                op0=mybir.AluOpType.is_equal,
            )

            # fused gather + W1: preact.T [KQ, BS]
            pre_ps = ps_pre.tile([KQ, BS], F32, tag="pre")
            nc.tensor.matmul(pre_ps, lhsT=gat_sb, rhs=oha_sb, start=True, stop=False)
            nc.tensor.matmul(pre_ps, lhsT=gct_sb, rhs=ohc_sb, start=False, stop=True)
            ht_sb = consts.tile([KQ, BS], F32R)
            nc.scalar.activation(
                out=ht_sb, in_=pre_ps, func=mybir.ActivationFunctionType.Tanh,
                bias=b1_sb, scale=1.0,
            )

            # softmax over NB bases, per 128-row tile
            coef_sb = []
            for m in range(MT):
                lg_ps = ps_pre.tile([128, NB + 1], F32, tag="pre")
                nc.tensor.matmul(
                    lg_ps, lhsT=ht_sb[:, m * 128:(m + 1) * 128], rhs=w2r_sb,
                    start=True, stop=True,
                )
                e_sb = work.tile([128, NB], F32, tag="e")
                nc.scalar.activation(
                    out=e_sb, in_=lg_ps[:, 0:NB],
                    func=mybir.ActivationFunctionType.Exp,
                )
                s_sb = work.tile([128, 1], F32, tag="s")
                nc.vector.reduce_sum(out=s_sb, in_=e_sb, axis=mybir.AxisListType.X)
                r_sb = work.tile([128, 1], F32, tag="r")
                nc.vector.reciprocal(out=r_sb, in_=s_sb)
                cf = consts.tile([128, NB], F32, name=f"coef{m}", tag=f"coef{m}")
                nc.vector.tensor_scalar(
                    out=cf, in0=e_sb, scalar1=r_sb, scalar2=None,
                    op0=mybir.AluOpType.mult,
                )
                coef_sb.append(cf)

            # ---- stage B: Y[m][b, o, j] = x_tile @ (W3j + b3r), k-accumulated
            y_ps = [
                ps_y.tile([128, OS, NB], F32, tag="y", name=f"y{m}")
                for m in range(MT)
            ]
            for half in range(2):
                for m in range(MT):
                    for k in range(half * H, (half + 1) * H):
                        nc.tensor.matmul(
                            y_ps[m],
                            lhsT=xall[:, k, m * 128:(m + 1) * 128],
                            rhs=wall[:, k, :].rearrange("p (o j) -> p o j", j=NB),
                            start=(k == 0), stop=(k == KT - 1),
                        )

            # ---- combine: out[b,o] = sum_j coef[b,j] * Y[b,o,j] ----
            out_sb = consts.tile([128, MT, OS], F32)
            for m in range(MT):
                cb = bass.AP(
                    tensor=coef_sb[m].tensor, offset=coef_sb[m].offset,
                    ap=[list(coef_sb[m].ap[0]), [0, OS], list(coef_sb[m].ap[1])],
                )
                tmp = work.tile([128, OS, NB], F32, tag="tmp")
                nc.vector.tensor_tensor(
                    out=tmp, in0=y_ps[m], in1=cb, op=mybir.AluOpType.mult,
                )
                nc.vector.reduce_sum(
                    out=out_sb[:, m, :], in_=tmp, axis=mybir.AxisListType.X,
                )
                nc.sync.dma_start(out=out[:, m * OS:(m + 1) * OS],
                                  in_=out_sb[:, m, :])

    nc.compile()
    return nc


def _get_nc():
    global _NC_CACHE
    if _NC_CACHE is None:
        _NC_CACHE = _build_nc()
    return _NC_CACHE


def _make_in_maps(x, idx_author, idx_citation, emb_author, emb_citation,
                  W1, b1, W2, W3, b3):
    f = np.float32
    x = np.asarray(x, dtype=f)
    W3r = np.asarray(W3, dtype=f).reshape(IN, OUT, NB)
    b3r = np.asarray(b3, dtype=f).reshape(IN, OUT)
    W1 = np.asarray(W1, dtype=f)

    # packed f32 smalls [64, 332]: embA.T | embC.T | W1a.T | W1c.T | W2.T | b1
    sf = np.concatenate([
        np.asarray(emb_author, dtype=f).T,
        np.asarray(emb_citation, dtype=f).T,
        W1[:, :EMB].T,
        W1[:, EMB:].T,
        np.asarray(W2, dtype=f).T,
        np.asarray(b1, dtype=f).reshape(KQ, 1),
    ], axis=1)
    sf = np.ascontiguousarray(sf)

    ia = np.asarray(idx_author).astype(ml_dtypes.bfloat16)
    ic = np.asarray(idx_citation).astype(ml_dtypes.bfloat16)
    ones_bs = np.ones((BS,), ml_dtypes.bfloat16)
    bw = np.stack([np.ones(VOCAB, f), -np.arange(VOCAB, dtype=f)]
                  ).astype(ml_dtypes.bfloat16)

    # per out-shard weight blocks, bias folded in, j innermost, k packed
    wc_blocks = []
    for oj in range(Q_O):
        cols = slice(oj * OS, (oj + 1) * OS)
        blk = W3r[:, cols, :] + b3r[:, cols, None]       # [IN, OS, NB]
        blk = blk.reshape(KT, 128, OS * NB).transpose(1, 0, 2)
        wc_blocks.append(np.ascontiguousarray(blk.reshape(128, KT * OS * NB)))

    # x.T per batch shard, k packed: [128, KT*BS]
    xt_shards = []
    for bi in range(P_B):
        xs = x[bi * BS:(bi + 1) * BS, :].T               # [IN, BS]
        xs = xs.reshape(KT, 128, BS).transpose(1, 0, 2)
        xt_shards.append(np.ascontiguousarray(xs.reshape(128, KT * BS)))

    in_maps = []
    for c in range(P_B * Q_O):
        bi, oj = c // Q_O, c % Q_O
        rows = slice(bi * BS, (bi + 1) * BS)
        sb16 = np.concatenate([
            np.stack([ia[rows], ones_bs]),
            np.stack([ic[rows], ones_bs]),
            bw,
        ], axis=1)
        in_maps.append({
            "xt": xt_shards[bi],
            "wc": wc_blocks[oj],
            "sf": sf,
            "sb": np.ascontiguousarray(sb16),
        })
    return in_maps


def kernel(x, idx_author, idx_citation, emb_author, emb_citation,
           W1, b1, W2, W3, b3):
    global LAST_RESULT
    nc = _get_nc()
    in_maps = _make_in_maps(x, idx_author, idx_citation, emb_author,
                            emb_citation, W1, b1, W2, W3, b3)
    res = run_bass_kernel_spmd(nc, in_maps, core_ids=list(range(P_B * Q_O)))
    LAST_RESULT = res
    out = np.empty((B, OUT), dtype=np.float32)
    for c in range(P_B * Q_O):
        bi, oj = c // Q_O, c % Q_O
        blk = res.results[c]["out"].reshape(128, MT, OS).transpose(1, 0, 2)
        out[bi * BS:(bi + 1) * BS, oj * OS:(oj + 1) * OS] = \
            blk.reshape(BS, OS)
    return out


# revision 10
# speedup vs baseline: 1.2041x; 1.0097x over previous
"""Trainium2 Bass kernel for a basis-customized linear layer.

Reference computation (B=1024, IN=OUT=512, EMB=64, KQ=64, NB=3, VOCAB=100):
    embs = concat(emb_author[idx_author], emb_citation[idx_citation])  # [B, 128]
    h    = tanh(embs @ W1.T + b1)                                      # [B, 64]
    coef = softmax(h @ W2.T)                                           # [B, 3]
    w    = (coef @ W3.T + b3).reshape(B, IN, OUT)
    out  = einsum('bi,bio->bo', x, w)                                  # [B, 512]

Key rewrites:
  (1) w[b] = sum_j coef[b,j]*W3j + b3r, so
      out = sum_j coef[:,j] * (x @ W3j) + x @ b3r
      -- 3 shared [512,512] matmuls + a per-sample weighted combine, instead
      of materializing the 1GB per-sample weight.
  (2) since softmax coefs sum to 1, the bias folds into every basis block:
      out = sum_j coef[:,j] * (x @ (W3j + b3r))
  (3) the embedding gather is a one-hot matmul (built from an iota-compare),
      fused with W1 via the precomputed per-vocab table G = emb @ W1half.T.

Sharding over 8 cores: batch 2-way x out-column 4-way. Each core gets
x.T[:, batch_half] (1MB) and its 3 basis column-blocks (0.75MB), computes
coef for its 512 rows on-device, and writes a [512, 128] output block.
All inputs are packed host-side into partition-major [128, *] layouts so each
is a single descriptor-per-partition DMA.
"""

import numpy as np
import ml_dtypes

import concourse.bass as bass
import concourse.tile as tile
from concourse import bacc, mybir
from concourse.bass_utils import run_bass_kernel_spmd

# Problem dims (hardcoded per contract)
B, IN, OUT = 1024, 512, 512
EMB, KQ, NB, VOCAB = 64, 64, 3, 100
P_B, Q_O = 2, 4            # batch shards x out-col shards = 8 cores
BS = B // P_B              # 512 batch rows per core
OS = OUT // Q_O            # 128 out cols per core
KT = IN // 128             # 4 contraction tiles
MT = BS // 128             # 4 batch tiles per core

F32 = mybir.dt.float32
F32R = mybir.dt.float32r
BF16 = mybir.dt.bfloat16

SB16 = 2 * BS + VOCAB      # packed bf16 input columns
SF32 = NB + 1              # packed f32 small-input columns (W2.T | b1)

LAST_RESULT = None         # BassKernelResults of the most recent run (for test.py)

_NC_CACHE = None


def _build_nc():
    nc = bacc.Bacc("TRN2", target_bir_lowering=False, debug=False,
                   num_devices=P_B * Q_O)

    xt = nc.dram_tensor("xt", [128, KT * BS], F32R, kind="ExternalInput")
    wc = nc.dram_tensor("wc", [128, KT * NB * OS], F32R, kind="ExternalInput")
    gg = nc.dram_tensor("gg", [VOCAB, 2 * KQ], F32R, kind="ExternalInput")
    sf = nc.dram_tensor("sf", [KQ, SF32], F32, kind="ExternalInput")
    sb = nc.dram_tensor("sb", [2, SB16], BF16, kind="ExternalInput")
    out = nc.dram_tensor("out", [128, MT * OS], F32, kind="ExternalOutput")

    with tile.TileContext(nc) as tc:
        with (
            tc.tile_pool(name="consts", bufs=1) as consts,
            tc.tile_pool(name="work", bufs=3) as work,
            tc.tile_pool(name="ps_idx", bufs=2, space="PSUM") as ps_idx,
            tc.tile_pool(name="ps_pre", bufs=2, space="PSUM") as ps_pre,
            tc.tile_pool(name="ps_y", bufs=4, space="PSUM") as ps_y,
        ):
            # ---- packed input loads (one descriptor-per-partition each) ----
            sb_sb = consts.tile([2, SB16], BF16)
            nc.sync.dma_start(out=sb_sb, in_=sb[:, :])
            sf_sb = consts.tile([KQ, SF32], F32)
            nc.scalar.dma_start(out=sf_sb, in_=sf[:, :])
            gg_sb = consts.tile([VOCAB, 2 * KQ], F32R)
            nc.scalar.dma_start(out=gg_sb, in_=gg[:, :])
            xall = consts.tile([128, KT, BS], F32R)
            wall = consts.tile([128, KT, NB * OS], F32R)
            xt_k = xt[:, :].rearrange("p (k n) -> p k n", k=KT)
            wc_k = wc[:, :].rearrange("p (k n) -> p k n", k=KT)
            for k in range(KT):
                nc.scalar.dma_start(out=wall[:, k, :], in_=wc_k[:, k, :])
                nc.sync.dma_start(out=xall[:, k, :], in_=xt_k[:, k, :])

            idxa_sb = sb_sb[:, 0:BS]
            idxc_sb = sb_sb[:, BS:2 * BS]
            bw_sb = sb_sb[:, 2 * BS:2 * BS + VOCAB]
            gat_sb = gg_sb[:, 0:KQ]
            gct_sb = gg_sb[:, KQ:2 * KQ]
            w2t_sb = sf_sb[:, 0:NB]

            # b1 routed through ACT so Tanh's bias dep is same-engine
            b1_sb = consts.tile([KQ, 1], F32)
            nc.scalar.copy(out=b1_sb, in_=sf_sb[:, NB:NB + 1])
            w2r_sb = consts.tile([KQ, NB + 1], F32R)
            nc.vector.memset(w2r_sb.bitcast(mybir.dt.uint32), 0)
            nc.vector.tensor_copy(out=w2r_sb[:, 0:NB], in_=w2t_sb)

            # ---- stage A: coef for all BS rows ----
            # one-hot via K=2 matmul: psum[v,b] = idx[b]*1 + 1*(-v), then ==0
            bca_ps = ps_idx.tile([VOCAB, BS], F32, tag="idx")
            nc.tensor.matmul(bca_ps, lhsT=bw_sb, rhs=idxa_sb, start=True, stop=True)
            oha_sb = consts.tile([VOCAB, BS], F32R)
            nc.vector.tensor_scalar(
                out=oha_sb, in0=bca_ps, scalar1=0.0, scalar2=None,
                op0=mybir.AluOpType.is_equal,
            )
            bcc_ps = ps_idx.tile([VOCAB, BS], F32, tag="idx")
            nc.tensor.matmul(bcc_ps, lhsT=bw_sb, rhs=idxc_sb, start=True, stop=True)
            ohc_sb = consts.tile([VOCAB, BS], F32R)
            nc.vector.tensor_scalar(
                out=ohc_sb, in0=bcc_ps, scalar1=0.0, scalar2=None,
                op0=mybir.AluOpType.is_equal,
            )

            # fused gather + W1: preact.T [KQ, BS]
            pre_ps = ps_pre.tile([KQ, BS], F32, tag="pre")
            nc.tensor.matmul(pre_ps, lhsT=gat_sb, rhs=oha_sb, start=True, stop=False)
            nc.tensor.matmul(pre_ps, lhsT=gct_sb, rhs=ohc_sb, start=False, stop=True)
            ht_sb = consts.tile([KQ, BS], F32R)
            nc.scalar.activation(
                out=ht_sb, in_=pre_ps, func=mybir.ActivationFunctionType.Tanh,
                bias=b1_sb, scale=1.0,
            )

            # softmax over NB bases, per 128-row tile
            coef_sb = []
            for m in range(MT):
                lg_ps = ps_pre.tile([128, NB + 1], F32, tag="pre")
                nc.tensor.matmul(
                    lg_ps, lhsT=ht_sb[:, m * 128:(m + 1) * 128], rhs=w2r_sb,
                    start=True, stop=True,
                )
                e_sb = work.tile([128, NB], F32, tag="e")
                nc.scalar.activation(
                    out=e_sb, in_=lg_ps[:, 0:NB],
                    func=mybir.ActivationFunctionType.Exp,
                )
                s_sb = work.tile([128, 1], F32, tag="s")
                nc.vector.reduce_sum(out=s_sb, in_=e_sb, axis=mybir.AxisListType.X)
                r_sb = work.tile([128, 1], F32, tag="r")
                nc.vector.reciprocal(out=r_sb, in_=s_sb)
                cf = consts.tile([128, NB], F32, name=f"coef{m}", tag=f"coef{m}")
                nc.vector.tensor_scalar(
                    out=cf, in0=e_sb, scalar1=r_sb, scalar2=None,
                    op0=mybir.AluOpType.mult,
                )
                coef_sb.append(cf)

            # ---- stage B: Y[m][b, o, j] = x_tile @ (W3j + b3r), k-accumulated
            y_ps = [
                ps_y.tile([128, OS, NB], F32, tag="y", name=f"y{m}")
                for m in range(MT)
            ]
            for k in range(KT):
                for m in range(MT):
                    nc.tensor.matmul(
                        y_ps[m],
                        lhsT=xall[:, k, m * 128:(m + 1) * 128],
                        rhs=wall[:, k, :].rearrange("p (o j) -> p o j", j=NB),
                        start=(k == 0), stop=(k == KT - 1),
                    )

            # ---- combine: out[b,o] = sum_j coef[b,j] * Y[b,o,j] ----
            out_sb = consts.tile([128, MT, OS], F32)
            for m in range(MT):
                cb = bass.AP(
                    tensor=coef_sb[m].tensor, offset=coef_sb[m].offset,
                    ap=[list(coef_sb[m].ap[0]), [0, OS], list(coef_sb[m].ap[1])],
                )
                tmp = work.tile([128, OS, NB], F32, tag="tmp")
                nc.vector.tensor_tensor(
                    out=tmp, in0=y_ps[m], in1=cb, op=mybir.AluOpType.mult,
                )
                nc.vector.reduce_sum(
                    out=out_sb[:, m, :], in_=tmp, axis=mybir.AxisListType.X,
                )
                nc.scalar.dma_start(out=out[:, m * OS:(m + 1) * OS],
                                     in_=out_sb[:, m, :])

    nc.compile()
    return nc


def _get_nc():
    global _NC_CACHE
    if _NC_CACHE is None:
        _NC_CACHE = _build_nc()
    return _NC_CACHE


def _make_in_maps(x, idx_author, idx_citation, emb_author, emb_citation,
                  W1, b1, W2, W3, b3):
    f = np.float32
    x = np.asarray(x, dtype=f)
    W3r = np.asarray(W3, dtype=f).reshape(IN, OUT, NB)
    b3r = np.asarray(b3, dtype=f).reshape(IN, OUT)
    W1 = np.asarray(W1, dtype=f)

    # param-only folds: per-vocab gather tables G = emb @ W1half.T [VOCAB, KQ]
    gg = np.concatenate([
        np.asarray(emb_author, dtype=f) @ W1[:, :EMB].T,
        np.asarray(emb_citation, dtype=f) @ W1[:, EMB:].T,
    ], axis=1)
    gg = np.ascontiguousarray(gg)
    # packed f32 smalls [64, 4]: W2.T | b1
    sf = np.ascontiguousarray(np.concatenate([
        np.asarray(W2, dtype=f).T,
        np.asarray(b1, dtype=f).reshape(KQ, 1),
    ], axis=1))

    ia = np.asarray(idx_author).astype(ml_dtypes.bfloat16)
    ic = np.asarray(idx_citation).astype(ml_dtypes.bfloat16)
    ones_bs = np.ones((BS,), ml_dtypes.bfloat16)
    bw = np.stack([np.ones(VOCAB, f), -np.arange(VOCAB, dtype=f)]
                  ).astype(ml_dtypes.bfloat16)

    # per out-shard weight blocks, bias folded in, j innermost, k packed
    wc_blocks = []
    for oj in range(Q_O):
        cols = slice(oj * OS, (oj + 1) * OS)
        blk = W3r[:, cols, :] + b3r[:, cols, None]       # [IN, OS, NB]
        blk = blk.reshape(KT, 128, OS * NB).transpose(1, 0, 2)
        wc_blocks.append(np.ascontiguousarray(blk.reshape(128, KT * OS * NB)))

    # x.T per batch shard, k packed: [128, KT*BS]
    xt_shards = []
    for bi in range(P_B):
        xs = x[bi * BS:(bi + 1) * BS, :].T               # [IN, BS]
        xs = xs.reshape(KT, 128, BS).transpose(1, 0, 2)
        xt_shards.append(np.ascontiguousarray(xs.reshape(128, KT * BS)))

    in_maps = []
    for c in range(P_B * Q_O):
        bi, oj = c // Q_O, c % Q_O
        rows = slice(bi * BS, (bi + 1) * BS)
        sb16 = np.concatenate([
            np.stack([ia[rows], ones_bs]),
            np.stack([ic[rows], ones_bs]),
            bw,
        ], axis=1)
        in_maps.append({
            "xt": xt_shards[bi],
            "wc": wc_blocks[oj],
            "gg": gg,
            "sf": sf,
            "sb": np.ascontiguousarray(sb16),
        })
    return in_maps


def kernel(x, idx_author, idx_citation, emb_author, emb_citation,
           W1, b1, W2, W3, b3):
    global LAST_RESULT
    nc = _get_nc()
    in_maps = _make_in_maps(x, idx_author, idx_citation, emb_author,
                            emb_citation, W1, b1, W2, W3, b3)
    res = run_bass_kernel_spmd(nc, in_maps, core_ids=list(range(P_B * Q_O)))
    LAST_RESULT = res
    out = np.empty((B, OUT), dtype=np.float32)
    for c in range(P_B * Q_O):
        bi, oj = c // Q_O, c % Q_O
        blk = res.results[c]["out"].reshape(128, MT, OS).transpose(1, 0, 2)
        out[bi * BS:(bi + 1) * BS, oj * OS:(oj + 1) * OS] = \
            blk.reshape(BS, OS)
    return out
